# revision 12
# baseline (speedup 1.0000x reference)
"""Trainium2 Bass kernel for nn_AaD_MAPU (retrieval kNN + KL attraction / dispersion loss).

Reference computation:
    softmax_out = softmax(predictions)                      [B,C]
    f_norm      = l2_normalize(features)                    [B,D]
    fb          = fea_bank with rows trg_idx <- f_norm      [N,D]
    sb          = score_bank with rows trg_idx <- softmax   [N,C]
    distance    = f_norm @ fb.T                             [B,N]
    idx         = top_k(distance, K+1); idx_near = idx[:,1:]
    score_near  = sb[idx_near]                              [B,K,C]
    loss        = sum(score_near * (log(score_near) - softmax[:,None,:])) / B
    neg_pred    = mean(rowsum(softmax @ softmax.T - diag))
    out         = loss + neg_pred

Device strategy (8 NeuronCores, bank rows sharded, d-major layout):
  - Pad bank to 100352 rows; core c owns rows [c*12544, (c+1)*12544).
  - The host ships each shard already transposed ([D, 12544] fp32, a pure
    layout change done while sharding) so the contraction dim lands on
    SBUF partitions with contiguous DMA.
  - float32r matmuls (full-rate fp32) vs pre-transposed f_norm.T
    accumulate PSUM fp32 [128b, 512j] tiles; a handful of zero warm-up
    matmuls during the first DMA bring the PE out of its cold p-state.
  - VectorE segmented reduce_max (8-wide segments) straight from PSUM
    -> per-row segment maxima [128, 1568] fp32 per batch chunk.
  - VectorE max8 + find_index8 per half (the first half fires while the
    matmul loop is still running) -> per-row top-8 (segment max, id) per
    half per 128-row chunk.
Host merges 8 cores x 16 candidates per row, resolves the argmax position
inside each 8-wide winning segment with a handful of fp32 dots, drops the
top-1 (reference drops idx[:,0]), gathers scores and reduces the loss.
"""

from contextlib import ExitStack

import numpy as np

import concourse.bass as bass
import concourse.tile as tile
from concourse import bacc, mybir
from concourse.bass_utils import run_bass_kernel_spmd

# Problem constants (hardcoded per contest rules).
B, D, N, C, K = 512, 512, 100000, 64, 5
EPS = 1e-12
NCORES = 8
NSHARD = 12544            # padded bank rows per core (98 * 128)
NPAD = NSHARD * NCORES    # 100352
SEG = 8                   # segment width for the two-level top-k
NSEG = NSHARD // SEG      # 1568 segments per core per row
BCH = 4                   # batch chunks of 128 rows
JT = 512                  # j-tile width
NJT = 25                  # 24 full tiles + one 256-wide tile
# segmax is split in parts; each part's top-8 extraction starts as
# soon as its segments are final, hiding most of the work under the matmuls
SPLIT_TILES = (9, 16, 22, NJT)        # part boundaries, in j-tiles
SPLIT_SEGS = (0, 576, 1024, 1408, NSEG)   # corresponding segment offsets
PARTS = 4
N_WARMUP = 10             # zero matmuls to warm the PE during the first DMA

_F32 = mybir.dt.float32
_F32R = mybir.dt.float32r
_U32 = mybir.dt.uint32

_cache = {}


def _build_module():
    nc = bacc.Bacc("TRN2", target_bir_lowering=False, debug=False,
                   num_devices=NCORES)
    # bank shard, transposed on host: [D, NSHARD] fp32 (float32r = same bits)
    fbt_d = nc.dram_tensor("fbt", [D, NSHARD], _F32R, kind="ExternalInput").ap()
    # f_norm.T packed on host as [dp, dc*B + b]
    fnt_d = nc.dram_tensor("fnt", [128, 4 * B], _F32R, kind="ExternalInput").ap()
    # top-8 per (128-row chunk, part): value and segment id packed together
    # ([..., 0, :] = fp32 value, [..., 1, :] = uint32 segment id as raw bits)
    cat_out = nc.dram_tensor("cat_out", [128, BCH, PARTS, 2, 8], _F32,
                             kind="ExternalOutput").ap()

    with tile.TileContext(nc) as tc, ExitStack() as ctx:
        const = ctx.enter_context(tc.tile_pool(name="const", bufs=1))
        fbt_pool = ctx.enter_context(tc.tile_pool(name="fbt", bufs=4))
        dp_pool = ctx.enter_context(tc.tile_pool(name="dp", bufs=4, space="PSUM"))
        out_pool = ctx.enter_context(tc.tile_pool(name="outs", bufs=1))

        # PE warm-up: harmless zero matmuls that run while the first DMAs land
        wu_sb = const.tile([128, JT], _F32)
        nc.gpsimd.memset(wu_sb[:], 0.0)
        wu_ps = dp_pool.tile([128, 2, JT], _F32, tag="dp")
        wu_r = wu_sb[:].bitcast(_F32R)
        for _ in range(N_WARMUP):
            nc.tensor.matmul(wu_ps[:, 0], lhsT=wu_r[:, :128], rhs=wu_r,
                             start=True, stop=True)

        # f_norm.T on the SWDGE queue, in parallel with the first bank tile
        fnt_sb = [const.tile([128, B], _F32R, name=f"fnt{dc}") for dc in range(4)]
        for dc in range(4):
            nc.gpsimd.dma_start(fnt_sb[dc][:], fnt_d[:, dc * B:(dc + 1) * B])

        segmax = const.tile([128, BCH, NSEG], _F32)
        cat = out_pool.tile([128, BCH, PARTS, 2, 8], _F32)

        def top8(bc, part):
            lo, hi = SPLIT_SEGS[part], SPLIT_SEGS[part + 1]
            sl = segmax[:, bc, lo:hi]
            nc.vector.max(out=cat[:, bc, part, 0], in_=sl)
            nc.vector.max_index(out=cat[:, bc, part, 1].bitcast(_U32),
                                in_max=cat[:, bc, part, 0], in_values=sl)

        for t in range(NJT):
            j0 = t * JT
            W = min(JT, NSHARD - j0)

            # bank tile in [d, j] layout: partition = d % 128, c = d // 128
            fbt = fbt_pool.tile([128, 4, JT], _F32R, tag="fbt")
            src = fbt_d[:, j0:j0 + W].rearrange("(c p) j -> p c j", p=128)
            if t == 0:
                # split the first tile so matmuls start as each chunk lands
                for dc in range(4):
                    nc.sync.dma_start(fbt[:, dc, :W], src[:, dc])
            else:
                nc.sync.dma_start(fbt[:, :, :W], src)

            for bcp in range(2):          # pairs of 128-row batch chunks
                dp = dp_pool.tile([128, 2, JT], _F32, tag="dp")
                for i in range(2):
                    bc = bcp * 2 + i
                    for dc in range(4):
                        nc.tensor.matmul(
                            dp[:, i, :W],
                            lhsT=fnt_sb[dc][:, bc * 128:(bc + 1) * 128],
                            rhs=fbt[:, dc, :W],
                            start=(dc == 0), stop=(dc == 3),
                        )
                nc.vector.tensor_reduce(
                    out=segmax[:, bcp * 2:bcp * 2 + 2,
                               t * (JT // SEG): t * (JT // SEG) + W // SEG],
                    in_=dp[:, :, :W].rearrange("p i (g e) -> p i g e", e=SEG),
                    axis=mybir.AxisListType.X,
                    op=mybir.AluOpType.max,
                )

            # spread finished parts' top-8 extraction across later tiles
            for part in range(PARTS - 1):
                if SPLIT_TILES[part] <= t < SPLIT_TILES[part] + BCH:
                    top8(t - SPLIT_TILES[part], part)

        for bc in range(BCH):
            top8(bc, PARTS - 1)
        nc.sync.dma_start(cat_out, cat[:])

    nc.compile()
    return nc


def _get_module():
    if "nc" not in _cache:
        _cache["nc"] = _build_module()
    return _cache["nc"]


def kernel(features, predictions, fea_bank, score_bank, trg_idx):
    features = np.asarray(features, dtype=np.float32)
    predictions = np.asarray(predictions, dtype=np.float32)
    fea_bank = np.asarray(fea_bank, dtype=np.float32)
    score_bank = np.asarray(score_bank, dtype=np.float32)
    trg_idx = np.asarray(trg_idx, dtype=np.int32)

    # ---- tiny host prologue (O(B*D)) ----
    sm = predictions - predictions.max(axis=1, keepdims=True)
    np.exp(sm, out=sm)
    sm /= sm.sum(axis=1, keepdims=True)                       # softmax_out [B,C]
    nrm = np.maximum(np.sqrt((features * features).sum(axis=1, keepdims=True)),
                     EPS)
    f_norm = features / nrm                                   # [B,D]

    # bank updates + padding
    fbp = np.zeros((NPAD, D), dtype=np.float32)
    fbp[:N] = fea_bank
    fbp[trg_idx] = f_norm
    sb = score_bank.copy()
    sb[trg_idx] = sm

    # f_norm.T packed as [dp, dc*B + b]
    fnt = np.ascontiguousarray(
        f_norm.T.reshape(4, 128, B).transpose(1, 0, 2).reshape(128, 4 * B))

    nc = _get_module()
    in_maps = [
        {"fbt": np.ascontiguousarray(fbp[c * NSHARD:(c + 1) * NSHARD].T),
         "fnt": fnt}
        for c in range(NCORES)
    ]
    res = run_bass_kernel_spmd(nc, in_maps, core_ids=list(range(NCORES)))

    # ---- host epilogue: merge candidates, resolve indices, loss ----
    # output is [128, BCH, PARTS, 2, 8]; row b = bc*128 + p
    CAND = PARTS * 8
    vals = np.empty((B, NCORES * CAND), np.float32)
    base = np.empty((B, NCORES * CAND), np.int64)
    part_off = np.asarray(SPLIT_SEGS[:PARTS], np.int64)[None, :, None]
    for c, r in enumerate(res.results):
        cat = np.ascontiguousarray(r["cat_out"].transpose(1, 0, 2, 3, 4))
        v = cat[:, :, :, 0].reshape(B, CAND)
        s = cat[:, :, :, 1].view(np.uint32).astype(np.int64)
        s = (s + part_off[None]).reshape(B, CAND)
        vals[:, c * CAND:(c + 1) * CAND] = v
        base[:, c * CAND:(c + 1) * CAND] = c * NSHARD + s * SEG

    TOP = 8  # resolve a couple extra candidates for tie-order safety
    order = np.argsort(-vals, axis=1, kind="stable")[:, :TOP]
    top_vals = np.take_along_axis(vals, order, axis=1)        # [B, TOP]
    top_base = np.take_along_axis(base, order, axis=1)        # [B, TOP]

    # resolve argmax position within each winning 8-wide segment (fp32 dots)
    rows = top_base[:, :, None] + np.arange(SEG, dtype=np.int64)[None, None, :]
    seg_vecs = fbp[rows.reshape(-1)].reshape(B, TOP, SEG, D)
    dots = np.einsum("rksd,rd->rks", seg_vecs, f_norm, optimize=True)
    pos = dots.argmax(axis=2)                                 # [B, TOP]
    top_idx = top_base + pos                                  # [B, TOP] global rows

    # order exactly like jax.lax.top_k: value desc, index asc on ties
    reorder = np.lexsort((top_idx, -top_vals), axis=1)
    top_idx = np.take_along_axis(top_idx, reorder, axis=1)

    idx_near = top_idx[:, 1:K + 1]                            # drop self slot 0
    score_near = sb[idx_near].astype(np.float64)              # [B,K,C]
    kl = score_near * (np.log(score_near) - sm[:, None, :].astype(np.float64))
    loss = kl.sum(axis=(1, 2)).mean()

    s64 = sm.astype(np.float64)
    neg_pred = (np.square(s64.sum(axis=0)).sum()
                - np.square(s64).sum()) / B

    return np.float32(loss + neg_pred)


# revision 21
# speedup vs baseline: 1.2025x; 1.2025x over previous
"""Trainium2 Bass kernel for nn_AaD_MAPU (retrieval kNN + KL attraction / dispersion loss).

Reference computation:
    softmax_out = softmax(predictions)                      [B,C]
    f_norm      = l2_normalize(features)                    [B,D]
    fb          = fea_bank with rows trg_idx <- f_norm      [N,D]
    sb          = score_bank with rows trg_idx <- softmax   [N,C]
    distance    = f_norm @ fb.T                             [B,N]
    idx         = top_k(distance, K+1); idx_near = idx[:,1:]
    score_near  = sb[idx_near]                              [B,K,C]
    loss        = sum(score_near * (log(score_near) - softmax[:,None,:])) / B
    neg_pred    = mean(rowsum(softmax @ softmax.T - diag))
    out         = loss + neg_pred

Device strategy (8 NeuronCores, bank rows sharded, d-major layout):
  - Pad bank to 100352 rows; core c owns rows [c*12544, (c+1)*12544).
  - The host ships each shard already transposed ([D, 12544] fp32, a pure
    layout change done while sharding) so the contraction dim lands on
    SBUF partitions with contiguous DMA.
  - float32r matmuls (full-rate fp32) vs pre-transposed f_norm.T
    accumulate PSUM fp32 [128b, 512j] tiles; a handful of zero warm-up
    matmuls during the first DMA bring the PE out of its cold p-state.
  - VectorE segmented reduce_max (8-wide segments) straight from PSUM
    -> per-row segment maxima [128, 1568] fp32 per batch chunk.
  - VectorE max8 + find_index8 per half (the first half fires while the
    matmul loop is still running) -> per-row top-8 (segment max, id) per
    half per 128-row chunk.
Host merges 8 cores x 16 candidates per row, resolves the argmax position
inside each 8-wide winning segment with a handful of fp32 dots, drops the
top-1 (reference drops idx[:,0]), gathers scores and reduces the loss.
"""

from contextlib import ExitStack

import numpy as np

import concourse.bass as bass
import concourse.tile as tile
from concourse import bacc, mybir
from concourse.bass_utils import run_bass_kernel_spmd

# Problem constants (hardcoded per contest rules).
B, D, N, C, K = 512, 512, 100000, 64, 5
EPS = 1e-12
NCORES = 8
NSHARD = 12544            # padded bank rows per core (98 * 128)
NPAD = NSHARD * NCORES    # 100352
SEG = 8                   # segment width for the two-level top-k
NSEG = NSHARD // SEG      # 1568 segments per core per row
BCH = 4                   # batch chunks of 128 rows
JT = 512                  # j-tile width
NJT = 25                  # 24 full tiles + one 256-wide tile
# segmax is split in parts; each part's top-8 extraction starts as
# soon as its segments are final, hiding most of the work under the matmuls
SPLIT_TILES = (9, 16, 22, NJT)        # part boundaries, in j-tiles
SPLIT_SEGS = (0, 576, 1024, 1408, NSEG)   # corresponding segment offsets
PARTS = 4
N_WARMUP = 16             # zero matmuls to warm the PE during the first DMA

_F32 = mybir.dt.float32
_F32R = mybir.dt.float32r
_U32 = mybir.dt.uint32

_cache = {}


def _build_module():
    nc = bacc.Bacc("TRN2", target_bir_lowering=False, debug=False,
                   num_devices=NCORES)
    # bank shard, transposed on host: [D, NSHARD] fp32 (float32r = same bits)
    fbt_d = nc.dram_tensor("fbt", [D, NSHARD], _F32R, kind="ExternalInput").ap()
    # f_norm.T packed on host as [dp, dc*B + b]
    fnt_d = nc.dram_tensor("fnt", [128, 4 * B], _F32R, kind="ExternalInput").ap()
    # top-8 per (128-row chunk, part): value and segment id
    val_out = nc.dram_tensor("val_out", [128, BCH, PARTS, 8], _F32,
                             kind="ExternalOutput").ap()
    idx_out = nc.dram_tensor("idx_out", [128, BCH, PARTS, 8], _U32,
                             kind="ExternalOutput").ap()

    with tile.TileContext(nc) as tc, ExitStack() as ctx:
        const = ctx.enter_context(tc.tile_pool(name="const", bufs=1))
        fbt_pool = ctx.enter_context(tc.tile_pool(name="fbt", bufs=4))
        dp_pool = ctx.enter_context(tc.tile_pool(name="dp", bufs=4, space="PSUM"))
        out_pool = ctx.enter_context(tc.tile_pool(name="outs", bufs=1))

        # PE warm-up: harmless zero matmuls that run while the first DMAs land
        wu_sb = const.tile([128, JT], _F32)
        nc.gpsimd.memset(wu_sb[:], 0.0)
        wu_ps = dp_pool.tile([128, 2, JT], _F32, tag="dp")
        wu_r = wu_sb[:].bitcast(_F32R)
        for _ in range(N_WARMUP):
            nc.tensor.matmul(wu_ps[:, 0], lhsT=wu_r[:, :128], rhs=wu_r,
                             start=True, stop=True)

        fnt_sb = [const.tile([128, B], _F32R, name=f"fnt{dc}") for dc in range(4)]
        for dc in range(4):
            nc.sync.dma_start(fnt_sb[dc][:], fnt_d[:, dc * B:(dc + 1) * B])

        segmax = const.tile([128, BCH, NSEG], _F32)
        vcat = out_pool.tile([128, BCH, PARTS, 8], _F32)
        icat = out_pool.tile([128, BCH, PARTS, 8], _U32)

        def top8(bc, part):
            lo, hi = SPLIT_SEGS[part], SPLIT_SEGS[part + 1]
            sl = segmax[:, bc, lo:hi]
            nc.vector.max(out=vcat[:, bc, part], in_=sl)
            nc.vector.max_index(out=icat[:, bc, part], in_max=vcat[:, bc, part],
                                in_values=sl)

        done = set()
        for t in range(NJT):
            j0 = t * JT
            W = min(JT, NSHARD - j0)

            # bank tile in [d, j] layout: partition = d % 128, c = d // 128
            fbt = fbt_pool.tile([128, 4, JT], _F32R, tag="fbt")
            nc.sync.dma_start(
                fbt[:, :, :W],
                fbt_d[:, j0:j0 + W].rearrange("(c p) j -> p c j", p=128),
            )

            for bcp in range(2):          # pairs of 128-row batch chunks
                dp = dp_pool.tile([128, 2, JT], _F32, tag="dp")
                for i in range(2):
                    bc = bcp * 2 + i
                    for dc in range(4):
                        nc.tensor.matmul(
                            dp[:, i, :W],
                            lhsT=fnt_sb[dc][:, bc * 128:(bc + 1) * 128],
                            rhs=fbt[:, dc, :W],
                            start=(dc == 0), stop=(dc == 3),
                        )
                nc.vector.tensor_reduce(
                    out=segmax[:, bcp * 2:bcp * 2 + 2,
                               t * (JT // SEG): t * (JT // SEG) + W // SEG],
                    in_=dp[:, :, :W].rearrange("p i (g e) -> p i g e", e=SEG),
                    axis=mybir.AxisListType.X,
                    op=mybir.AluOpType.max,
                )

            # spread finished parts' top-8 extraction across later tiles
            for part in range(PARTS - 1):
                if SPLIT_TILES[part] <= t < SPLIT_TILES[part] + BCH:
                    done.add((t - SPLIT_TILES[part], part))
                    top8(t - SPLIT_TILES[part], part)

        for part in range(PARTS):
            for bc in range(BCH):
                if (bc, part) not in done:
                    top8(bc, part)
        nc.sync.dma_start(val_out, vcat[:])
        nc.sync.dma_start(idx_out, icat[:])

    nc.compile()
    return nc


def _get_module():
    if "nc" not in _cache:
        _cache["nc"] = _build_module()
    return _cache["nc"]


def kernel(features, predictions, fea_bank, score_bank, trg_idx):
    features = np.asarray(features, dtype=np.float32)
    predictions = np.asarray(predictions, dtype=np.float32)
    fea_bank = np.asarray(fea_bank, dtype=np.float32)
    score_bank = np.asarray(score_bank, dtype=np.float32)
    trg_idx = np.asarray(trg_idx, dtype=np.int32)

    # ---- tiny host prologue (O(B*D)) ----
    sm = predictions - predictions.max(axis=1, keepdims=True)
    np.exp(sm, out=sm)
    sm /= sm.sum(axis=1, keepdims=True)                       # softmax_out [B,C]
    nrm = np.maximum(np.sqrt((features * features).sum(axis=1, keepdims=True)),
                     EPS)
    f_norm = features / nrm                                   # [B,D]

    # bank updates + padding
    fbp = np.zeros((NPAD, D), dtype=np.float32)
    fbp[:N] = fea_bank
    fbp[trg_idx] = f_norm
    sb = score_bank.copy()
    sb[trg_idx] = sm

    # f_norm.T packed as [dp, dc*B + b]
    fnt = np.ascontiguousarray(
        f_norm.T.reshape(4, 128, B).transpose(1, 0, 2).reshape(128, 4 * B))

    nc = _get_module()
    in_maps = [
        {"fbt": np.ascontiguousarray(fbp[c * NSHARD:(c + 1) * NSHARD].T),
         "fnt": fnt}
        for c in range(NCORES)
    ]
    res = run_bass_kernel_spmd(nc, in_maps, core_ids=list(range(NCORES)))

    # ---- host epilogue: merge candidates, resolve indices, loss ----
    # output is [128, BCH, PARTS, 2, 8]; row b = bc*128 + p
    CAND = PARTS * 8
    vals = np.empty((B, NCORES * CAND), np.float32)
    base = np.empty((B, NCORES * CAND), np.int64)
    part_off = np.asarray(SPLIT_SEGS[:PARTS], np.int64)[None, :, None]
    for c, r in enumerate(res.results):
        v = r["val_out"].transpose(1, 0, 2, 3).reshape(B, CAND)
        s = r["idx_out"].astype(np.int64).transpose(1, 0, 2, 3)
        s = (s + part_off[None]).reshape(B, CAND)
        vals[:, c * CAND:(c + 1) * CAND] = v
        base[:, c * CAND:(c + 1) * CAND] = c * NSHARD + s * SEG

    TOP = 8  # resolve a couple extra candidates for tie-order safety
    order = np.argsort(-vals, axis=1, kind="stable")[:, :TOP]
    top_vals = np.take_along_axis(vals, order, axis=1)        # [B, TOP]
    top_base = np.take_along_axis(base, order, axis=1)        # [B, TOP]

    # resolve argmax position within each winning 8-wide segment (fp32 dots)
    rows = top_base[:, :, None] + np.arange(SEG, dtype=np.int64)[None, None, :]
    seg_vecs = fbp[rows.reshape(-1)].reshape(B, TOP, SEG, D)
    dots = np.einsum("rksd,rd->rks", seg_vecs, f_norm, optimize=True)
    pos = dots.argmax(axis=2)                                 # [B, TOP]
    top_idx = top_base + pos                                  # [B, TOP] global rows

    # order exactly like jax.lax.top_k: value desc, index asc on ties
    reorder = np.lexsort((top_idx, -top_vals), axis=1)
    top_idx = np.take_along_axis(top_idx, reorder, axis=1)

    idx_near = top_idx[:, 1:K + 1]                            # drop self slot 0
    score_near = sb[idx_near].astype(np.float64)              # [B,K,C]
    kl = score_near * (np.log(score_near) - sm[:, None, :].astype(np.float64))
    loss = kl.sum(axis=(1, 2)).mean()

    s64 = sm.astype(np.float64)
    neg_pred = (np.square(s64.sum(axis=0)).sum()
                - np.square(s64).sum()) / B

    return np.float32(loss + neg_pred)


# revision 27
# speedup vs baseline: 1.4552x; 1.2102x over previous
"""Trainium2 Bass kernel for nn_AaD_MAPU (retrieval kNN + KL attraction / dispersion loss).

Reference computation:
    softmax_out = softmax(predictions)                      [B,C]
    f_norm      = l2_normalize(features)                    [B,D]
    fb          = fea_bank with rows trg_idx <- f_norm      [N,D]
    sb          = score_bank with rows trg_idx <- softmax   [N,C]
    distance    = f_norm @ fb.T                             [B,N]
    idx         = top_k(distance, K+1); idx_near = idx[:,1:]
    score_near  = sb[idx_near]                              [B,K,C]
    loss        = sum(score_near * (log(score_near) - softmax[:,None,:])) / B
    neg_pred    = mean(rowsum(softmax @ softmax.T - diag))
    out         = loss + neg_pred

Device strategy (8 NeuronCores, bank rows sharded, d-major layout):
  - Pad bank to 100352 rows; core c owns rows [c*12544, (c+1)*12544).
  - The host ships each shard already transposed ([D, 12544] fp32, a pure
    layout change done while sharding) so the contraction dim lands on
    SBUF partitions with contiguous DMA.
  - float32r matmuls (full-rate fp32) vs pre-transposed f_norm.T
    accumulate PSUM fp32 [128b, 512j] tiles; a handful of zero warm-up
    matmuls during the first DMA bring the PE out of its cold p-state.
  - VectorE segmented reduce_max (8-wide segments) straight from PSUM
    -> per-row segment maxima [128, 1568] fp32 per batch chunk.
  - VectorE max8 + find_index8 per half (the first half fires while the
    matmul loop is still running) -> per-row top-8 (segment max, id) per
    half per 128-row chunk.
Host merges 8 cores x 16 candidates per row, resolves the argmax position
inside each 8-wide winning segment with a handful of fp32 dots, drops the
top-1 (reference drops idx[:,0]), gathers scores and reduces the loss.
"""

from contextlib import ExitStack

import numpy as np

import concourse.bass as bass
import concourse.tile as tile
from concourse import bacc, mybir
from concourse.bass_utils import run_bass_kernel_spmd

# Problem constants (hardcoded per contest rules).
B, D, N, C, K = 512, 512, 100000, 64, 5
EPS = 1e-12
NCORES = 8
NSHARD = 12544            # padded bank rows per core (98 * 128)
NPAD = NSHARD * NCORES    # 100352
SEG = 8                   # segment width for the two-level top-k
NSEG = NSHARD // SEG      # 1568 segments per core per row
BCH = 4                   # batch chunks of 128 rows
JT = 512                  # j-tile width
NJT = 25                  # 24 full tiles + one 256-wide tile
# segmax is split in parts; each part's top-8 extraction starts as
# soon as its segments are final, hiding most of the work under the matmuls
SPLIT_TILES = (9, 16, 22, NJT)        # part boundaries, in j-tiles
SPLIT_SEGS = (0, 576, 1024, 1408, NSEG)   # corresponding segment offsets
PARTS = 4
N_WARMUP = 16             # zero matmuls to warm the PE during the first DMA
FSCALE = 16.0             # f_norm pre-scale so fp8 quantization is well-conditioned

_F32 = mybir.dt.float32
_FP8 = mybir.dt.float8e4
_U32 = mybir.dt.uint32

_cache = {}


def _build_module():
    nc = bacc.Bacc("TRN2", target_bir_lowering=False, debug=False,
                   num_devices=NCORES)
    # bank shard, transposed + fp8-cast on host: [D, NSHARD]
    fbt_d = nc.dram_tensor("fbt", [D, NSHARD], _FP8, kind="ExternalInput").ap()
    # f_norm.T (pre-scaled, fp8) packed on host as [dp, dc*B + b]
    fnt_d = nc.dram_tensor("fnt", [128, 4 * B], _FP8, kind="ExternalInput").ap()
    # top-8 per (128-row chunk, part): value and segment id
    val_out = nc.dram_tensor("val_out", [128, BCH, PARTS, 8], _F32,
                             kind="ExternalOutput").ap()
    idx_out = nc.dram_tensor("idx_out", [128, BCH, PARTS, 8], _U32,
                             kind="ExternalOutput").ap()

    with tile.TileContext(nc) as tc, ExitStack() as ctx:
        const = ctx.enter_context(tc.tile_pool(name="const", bufs=1))
        fbt_pool = ctx.enter_context(tc.tile_pool(name="fbt", bufs=4))
        dp_pool = ctx.enter_context(tc.tile_pool(name="dp", bufs=4, space="PSUM"))
        out_pool = ctx.enter_context(tc.tile_pool(name="outs", bufs=1))

        # PE warm-up: harmless zero matmuls that run while the first DMAs land
        wu_sb = const.tile([128, JT], _F32)
        nc.gpsimd.memset(wu_sb[:], 0.0)
        wu_ps = dp_pool.tile([128, 2, JT], _F32, tag="dp")
        wu_r = wu_sb[:].bitcast(_FP8).rearrange("p (c j) -> p c j", c=4)
        for _ in range(N_WARMUP):
            nc.tensor.matmul(wu_ps[:, 0], lhsT=wu_r[:, 0:2, :128], rhs=wu_r[:, 0:2],
                             start=True, stop=True,
                             perf_mode=mybir.MatmulPerfMode.DoubleRow)

        fnt_sb = const.tile([128, 4, B], _FP8)
        nc.sync.dma_start(fnt_sb[:], fnt_d.rearrange("p (c b) -> p c b", c=4))

        segmax = const.tile([128, BCH, NSEG], _F32)
        vcat = out_pool.tile([128, BCH, PARTS, 8], _F32)
        icat = out_pool.tile([128, BCH, PARTS, 8], _U32)

        def top8(bc, part):
            lo, hi = SPLIT_SEGS[part], SPLIT_SEGS[part + 1]
            sl = segmax[:, bc, lo:hi]
            nc.vector.max(out=vcat[:, bc, part], in_=sl)
            nc.vector.max_index(out=icat[:, bc, part], in_max=vcat[:, bc, part],
                                in_values=sl)

        done = set()
        for t in range(NJT):
            j0 = t * JT
            W = min(JT, NSHARD - j0)

            # bank tile in [d, j] layout: partition = d % 128, c = d // 128
            fbt = fbt_pool.tile([128, 4, JT], _FP8, tag="fbt")
            nc.sync.dma_start(
                fbt[:, :, :W],
                fbt_d[:, j0:j0 + W].rearrange("(c p) j -> p c j", p=128),
            )

            for bcp in range(2):          # pairs of 128-row batch chunks
                dp = dp_pool.tile([128, 2, JT], _F32, tag="dp")
                for i in range(2):
                    bc = bcp * 2 + i
                    for h in range(2):    # DoubleRow: two d-chunks per matmul
                        nc.tensor.matmul(
                            dp[:, i, :W],
                            lhsT=fnt_sb[:, 2 * h:2 * h + 2, bc * 128:(bc + 1) * 128],
                            rhs=fbt[:, 2 * h:2 * h + 2, :W],
                            start=(h == 0), stop=(h == 1),
                            perf_mode=mybir.MatmulPerfMode.DoubleRow,
                        )
                nc.vector.tensor_reduce(
                    out=segmax[:, bcp * 2:bcp * 2 + 2,
                               t * (JT // SEG): t * (JT // SEG) + W // SEG],
                    in_=dp[:, :, :W].rearrange("p i (g e) -> p i g e", e=SEG),
                    axis=mybir.AxisListType.X,
                    op=mybir.AluOpType.max,
                )

            # spread finished parts' top-8 extraction across later tiles
            for part in range(PARTS - 1):
                if SPLIT_TILES[part] <= t < SPLIT_TILES[part] + BCH:
                    done.add((t - SPLIT_TILES[part], part))
                    top8(t - SPLIT_TILES[part], part)

        for part in range(PARTS):
            for bc in range(BCH):
                if (bc, part) not in done:
                    top8(bc, part)
        nc.sync.dma_start(val_out, vcat[:])
        nc.sync.dma_start(idx_out, icat[:])

    nc.compile()
    return nc


def _get_module():
    if "nc" not in _cache:
        _cache["nc"] = _build_module()
    return _cache["nc"]


def kernel(features, predictions, fea_bank, score_bank, trg_idx):
    features = np.asarray(features, dtype=np.float32)
    predictions = np.asarray(predictions, dtype=np.float32)
    fea_bank = np.asarray(fea_bank, dtype=np.float32)
    score_bank = np.asarray(score_bank, dtype=np.float32)
    trg_idx = np.asarray(trg_idx, dtype=np.int32)

    # ---- tiny host prologue (O(B*D)) ----
    sm = predictions - predictions.max(axis=1, keepdims=True)
    np.exp(sm, out=sm)
    sm /= sm.sum(axis=1, keepdims=True)                       # softmax_out [B,C]
    nrm = np.maximum(np.sqrt((features * features).sum(axis=1, keepdims=True)),
                     EPS)
    f_norm = features / nrm                                   # [B,D]

    # bank updates + padding
    fbp = np.zeros((NPAD, D), dtype=np.float32)
    fbp[:N] = fea_bank
    fbp[trg_idx] = f_norm
    sb = score_bank.copy()
    sb[trg_idx] = sm

    # f_norm.T (pre-scaled for fp8 conditioning) packed as [dp, dc*B + b]
    import ml_dtypes
    fp8 = ml_dtypes.float8_e4m3
    fnt = np.ascontiguousarray(
        (f_norm.T * FSCALE).reshape(4, 128, B).transpose(1, 0, 2)
        .reshape(128, 4 * B)).astype(fp8)

    nc = _get_module()
    in_maps = [
        {"fbt": np.ascontiguousarray(
             fbp[c * NSHARD:(c + 1) * NSHARD].T).astype(fp8),
         "fnt": fnt}
        for c in range(NCORES)
    ]
    res = run_bass_kernel_spmd(nc, in_maps, core_ids=list(range(NCORES)))

    # ---- host epilogue: merge candidates, resolve indices, loss ----
    # output is [128, BCH, PARTS, 2, 8]; row b = bc*128 + p
    CAND = PARTS * 8
    vals = np.empty((B, NCORES * CAND), np.float32)
    base = np.empty((B, NCORES * CAND), np.int64)
    part_off = np.asarray(SPLIT_SEGS[:PARTS], np.int64)[None, :, None]
    for c, r in enumerate(res.results):
        v = r["val_out"].transpose(1, 0, 2, 3).reshape(B, CAND)
        s = r["idx_out"].astype(np.int64).transpose(1, 0, 2, 3)
        s = (s + part_off[None]).reshape(B, CAND)
        vals[:, c * CAND:(c + 1) * CAND] = v
        base[:, c * CAND:(c + 1) * CAND] = c * NSHARD + s * SEG

    # preselect by the (fp8-precision) device values, then re-rank the short
    # list with exact fp32 dot products so fp8 noise cannot affect the result
    TOP = 12
    order = np.argsort(-vals, axis=1, kind="stable")[:, :TOP]
    top_base = np.take_along_axis(base, order, axis=1)        # [B, TOP]

    rows = top_base[:, :, None] + np.arange(SEG, dtype=np.int64)[None, None, :]
    seg_vecs = fbp[rows.reshape(-1)].reshape(B, TOP, SEG, D)
    dots = np.einsum("rksd,rd->rks", seg_vecs, f_norm, optimize=True)
    pos = dots.argmax(axis=2)                                 # [B, TOP]
    val32 = dots.max(axis=2)                                  # [B, TOP] fp32 seg max
    top_idx = top_base + pos                                  # [B, TOP] global rows

    # order exactly like jax.lax.top_k: value desc, index asc on ties
    reorder = np.lexsort((top_idx, -val32), axis=1)
    top_idx = np.take_along_axis(top_idx, reorder, axis=1)

    idx_near = top_idx[:, 1:K + 1]                            # drop self slot 0
    score_near = sb[idx_near].astype(np.float64)              # [B,K,C]
    kl = score_near * (np.log(score_near) - sm[:, None, :].astype(np.float64))
    loss = kl.sum(axis=(1, 2)).mean()

    s64 = sm.astype(np.float64)
    neg_pred = (np.square(s64.sum(axis=0)).sum()
                - np.square(s64).sum()) / B

    return np.float32(loss + neg_pred)


# revision 28
# speedup vs baseline: 1.8590x; 1.2775x over previous
"""Trainium2 Bass kernel for nn_AaD_MAPU (retrieval kNN + KL attraction / dispersion loss).

Reference computation:
    softmax_out = softmax(predictions)                      [B,C]
    f_norm      = l2_normalize(features)                    [B,D]
    fb          = fea_bank with rows trg_idx <- f_norm      [N,D]
    sb          = score_bank with rows trg_idx <- softmax   [N,C]
    distance    = f_norm @ fb.T                             [B,N]
    idx         = top_k(distance, K+1); idx_near = idx[:,1:]
    score_near  = sb[idx_near]                              [B,K,C]
    loss        = sum(score_near * (log(score_near) - softmax[:,None,:])) / B
    neg_pred    = mean(rowsum(softmax @ softmax.T - diag))
    out         = loss + neg_pred

Device strategy (8 NeuronCores, bank rows sharded, d-major layout):
  - Pad bank to 100352 rows; core c owns rows [c*12544, (c+1)*12544).
  - The host ships each shard transposed ([D, 12544]) and fp8-e4m3 cast
    (f_norm side pre-scaled x16 so unit-norm entries stay well above the
    fp8 subnormal range; a uniform scale cannot change the ranking).
  - fp8 DoubleRow matmuls (2 d-chunks per instruction, 0.5 cyc/row)
    accumulate PSUM fp32 [128b, 512j] distance tiles.
  - ScalarE copies each PSUM tile to SBUF bf16; VectorE keeps a running
    elementwise max ("comb max") over the j-tiles in bf16 2x mode:
    comb[b, e] = max_t distance[b, t*512 + e], e in [0, 512).
  - VectorE max8 + find_index8 per 128-row chunk -> top-8 (comb max, e).
Host merges 8 cores x 8 candidate combs per row, recomputes each winning
comb's ~25 member distances in exact fp32 (so fp8/bf16 noise cannot affect
the final selection), takes the top-2 per comb (covers two neighbours
landing in one comb), re-ranks, drops the top-1 (the reference drops
idx[:,0]), gathers scores and reduces the loss.
"""

from contextlib import ExitStack

import numpy as np

import concourse.bass as bass
import concourse.tile as tile
from concourse import bacc, mybir
from concourse.bass_utils import run_bass_kernel_spmd

# Problem constants (hardcoded per contest rules).
B, D, N, C, K = 512, 512, 100000, 64, 5
EPS = 1e-12
NCORES = 8
NSHARD = 12544            # padded bank rows per core (98 * 128)
NPAD = NSHARD * NCORES    # 100352
BCH = 4                   # batch chunks of 128 rows
JT = 512                  # j-tile width == comb count per core
NJT = 25                  # 24 full tiles + one 256-wide tile
N_WARMUP = 8              # zero matmuls to warm the PE during the first DMA
FSCALE = 16.0             # f_norm pre-scale so fp8 quantization is well-conditioned

_F32 = mybir.dt.float32
_BF16 = mybir.dt.bfloat16
_FP8 = mybir.dt.float8e4
_U32 = mybir.dt.uint32

_cache = {}


def _build_module():
    nc = bacc.Bacc("TRN2", target_bir_lowering=False, debug=False,
                   num_devices=NCORES)
    # bank shard, transposed + fp8-cast on host: [D, NSHARD]
    fbt_d = nc.dram_tensor("fbt", [D, NSHARD], _FP8, kind="ExternalInput").ap()
    # f_norm.T (pre-scaled, fp8) packed on host as [dp, dc*B + b]
    fnt_d = nc.dram_tensor("fnt", [128, 4 * B], _FP8, kind="ExternalInput").ap()
    # top-8 combs per 128-row chunk: value and comb id
    val_out = nc.dram_tensor("val_out", [128, BCH, 8], _F32,
                             kind="ExternalOutput").ap()
    idx_out = nc.dram_tensor("idx_out", [128, BCH, 8], _U32,
                             kind="ExternalOutput").ap()

    with tile.TileContext(nc) as tc, ExitStack() as ctx:
        const = ctx.enter_context(tc.tile_pool(name="const", bufs=1))
        fbt_pool = ctx.enter_context(tc.tile_pool(name="fbt", bufs=4))
        dp_pool = ctx.enter_context(tc.tile_pool(name="dp", bufs=3, space="PSUM"))
        tmp_pool = ctx.enter_context(tc.tile_pool(name="tmp", bufs=3))
        out_pool = ctx.enter_context(tc.tile_pool(name="outs", bufs=1))

        # PE warm-up: harmless zero matmuls that run while the first DMAs land
        wu_sb = const.tile([128, JT], _F32)
        nc.gpsimd.memset(wu_sb[:], 0.0)
        wu_ps = dp_pool.tile([128, 2, JT], _F32, tag="dp")
        wu_r = wu_sb[:].bitcast(_FP8).rearrange("p (c j) -> p c j", c=4)
        for _ in range(N_WARMUP):
            nc.tensor.matmul(wu_ps[:, 0], lhsT=wu_r[:, 0:2, :128], rhs=wu_r[:, 0:2],
                             start=True, stop=True,
                             perf_mode=mybir.MatmulPerfMode.DoubleRow)

        fnt_sb = const.tile([128, 4, B], _FP8)
        nc.sync.dma_start(fnt_sb[:], fnt_d.rearrange("p (c b) -> p c b", c=4))

        # running comb maxima, bf16: one [128, 2, JT] tile per pair of chunks
        acc = [const.tile([128, 2, JT], _BF16, name=f"acc{i}") for i in range(2)]
        vcat = out_pool.tile([128, BCH, 8], _F32)
        icat = out_pool.tile([128, BCH, 8], _U32)

        for t in range(NJT):
            j0 = t * JT
            W = min(JT, NSHARD - j0)

            # bank tile in [d, j] layout: partition = d % 128, c = d // 128
            fbt = fbt_pool.tile([128, 4, JT], _FP8, tag="fbt")
            nc.sync.dma_start(
                fbt[:, :, :W],
                fbt_d[:, j0:j0 + W].rearrange("(c p) j -> p c j", p=128),
            )

            for bcp in range(2):          # pairs of 128-row batch chunks
                dp = dp_pool.tile([128, 2, JT], _F32, tag="dp")
                for i in range(2):
                    bc = bcp * 2 + i
                    for h in range(2):    # DoubleRow: two d-chunks per matmul
                        nc.tensor.matmul(
                            dp[:, i, :W],
                            lhsT=fnt_sb[:, 2 * h:2 * h + 2, bc * 128:(bc + 1) * 128],
                            rhs=fbt[:, 2 * h:2 * h + 2, :W],
                            start=(h == 0), stop=(h == 1),
                            perf_mode=mybir.MatmulPerfMode.DoubleRow,
                        )
                if t == 0:
                    nc.scalar.copy(out=acc[bcp][:, :, :W], in_=dp[:, :, :W])
                else:
                    tmp = tmp_pool.tile([128, 2, JT], _BF16, tag="tmp")
                    nc.scalar.copy(out=tmp[:, :, :W], in_=dp[:, :, :W])
                    nc.vector.tensor_max(acc[bcp][:, :, :W],
                                         acc[bcp][:, :, :W], tmp[:, :, :W])

        for bc in range(BCH):
            sl = acc[bc // 2][:, bc % 2]
            nc.vector.max(out=vcat[:, bc], in_=sl)
            nc.vector.max_index(out=icat[:, bc], in_max=vcat[:, bc], in_values=sl)
        nc.sync.dma_start(val_out, vcat[:])
        nc.sync.dma_start(idx_out, icat[:])

    nc.compile()
    return nc


def _get_module():
    if "nc" not in _cache:
        _cache["nc"] = _build_module()
    return _cache["nc"]


def kernel(features, predictions, fea_bank, score_bank, trg_idx):
    features = np.asarray(features, dtype=np.float32)
    predictions = np.asarray(predictions, dtype=np.float32)
    fea_bank = np.asarray(fea_bank, dtype=np.float32)
    score_bank = np.asarray(score_bank, dtype=np.float32)
    trg_idx = np.asarray(trg_idx, dtype=np.int32)

    # ---- tiny host prologue (O(B*D)) ----
    sm = predictions - predictions.max(axis=1, keepdims=True)
    np.exp(sm, out=sm)
    sm /= sm.sum(axis=1, keepdims=True)                       # softmax_out [B,C]
    nrm = np.maximum(np.sqrt((features * features).sum(axis=1, keepdims=True)),
                     EPS)
    f_norm = features / nrm                                   # [B,D]

    # bank updates + padding
    fbp = np.zeros((NPAD, D), dtype=np.float32)
    fbp[:N] = fea_bank
    fbp[trg_idx] = f_norm
    sb = score_bank.copy()
    sb[trg_idx] = sm

    # f_norm.T (pre-scaled for fp8 conditioning) packed as [dp, dc*B + b]
    import ml_dtypes
    fp8 = ml_dtypes.float8_e4m3
    fnt = np.ascontiguousarray(
        (f_norm.T * FSCALE).reshape(4, 128, B).transpose(1, 0, 2)
        .reshape(128, 4 * B)).astype(fp8)

    nc = _get_module()
    in_maps = [
        {"fbt": np.ascontiguousarray(
             fbp[c * NSHARD:(c + 1) * NSHARD].T).astype(fp8),
         "fnt": fnt}
        for c in range(NCORES)
    ]
    res = run_bass_kernel_spmd(nc, in_maps, core_ids=list(range(NCORES)))

    # ---- host epilogue: merge candidate combs, resolve in fp32, loss ----
    # outputs are [128, BCH, 8]; row b = bc*128 + p
    vals = np.empty((B, NCORES * 8), np.float32)
    core = np.empty((B, NCORES * 8), np.int64)
    comb = np.empty((B, NCORES * 8), np.int64)
    for c, r in enumerate(res.results):
        vals[:, c * 8:(c + 1) * 8] = \
            r["val_out"].transpose(1, 0, 2).reshape(B, 8)
        comb[:, c * 8:(c + 1) * 8] = \
            r["idx_out"].astype(np.int64).transpose(1, 0, 2).reshape(B, 8)
        core[:, c * 8:(c + 1) * 8] = c

    # preselect by the (fp8/bf16-precision) device values, then re-rank the
    # short list with exact fp32 dots so quantization noise cannot leak in
    TOP = 12
    order = np.argsort(-vals, axis=1, kind="stable")[:, :TOP]
    top_core = np.take_along_axis(core, order, axis=1)        # [B, TOP]
    top_comb = np.take_along_axis(comb, order, axis=1)        # [B, TOP]

    tt = np.arange(NJT, dtype=np.int64)[None, None, :]
    pos_local = top_comb[:, :, None] + tt * JT                # [B, TOP, NJT]
    valid = pos_local < NSHARD
    rows = top_core[:, :, None] * NSHARD + np.minimum(pos_local, NSHARD - 1)
    vecs = fbp[rows.reshape(-1)].reshape(B, TOP, NJT, D)
    dots = np.einsum("rktd,rd->rkt", vecs, f_norm, optimize=True)
    dots = np.where(valid & (rows < N), dots, np.float32(-np.inf))

    # top-2 member rows per comb (two neighbours may share one comb)
    p2 = np.argsort(-dots, axis=2)[:, :, :2]                  # [B, TOP, 2]
    v2 = np.take_along_axis(dots, p2, axis=2).reshape(B, 2 * TOP)
    i2 = np.take_along_axis(rows, p2, axis=2).reshape(B, 2 * TOP)

    # order exactly like jax.lax.top_k: value desc, index asc on ties
    reorder = np.lexsort((i2, -v2), axis=1)
    top_idx = np.take_along_axis(i2, reorder, axis=1)

    idx_near = top_idx[:, 1:K + 1]                            # drop self slot 0
    score_near = sb[idx_near].astype(np.float64)              # [B,K,C]
    kl = score_near * (np.log(score_near) - sm[:, None, :].astype(np.float64))
    loss = kl.sum(axis=(1, 2)).mean()

    s64 = sm.astype(np.float64)
    neg_pred = (np.square(s64.sum(axis=0)).sum()
                - np.square(s64).sum()) / B

    return np.float32(loss + neg_pred)


# revision 30
# speedup vs baseline: 1.8845x; 1.0137x over previous
"""Trainium2 Bass kernel for nn_AaD_MAPU (retrieval kNN + KL attraction / dispersion loss).

Reference computation:
    softmax_out = softmax(predictions)                      [B,C]
    f_norm      = l2_normalize(features)                    [B,D]
    fb          = fea_bank with rows trg_idx <- f_norm      [N,D]
    sb          = score_bank with rows trg_idx <- softmax   [N,C]
    distance    = f_norm @ fb.T                             [B,N]
    idx         = top_k(distance, K+1); idx_near = idx[:,1:]
    score_near  = sb[idx_near]                              [B,K,C]
    loss        = sum(score_near * (log(score_near) - softmax[:,None,:])) / B
    neg_pred    = mean(rowsum(softmax @ softmax.T - diag))
    out         = loss + neg_pred

Device strategy (8 NeuronCores, bank rows sharded, d-major layout):
  - Pad bank to 100352 rows; core c owns rows [c*12544, (c+1)*12544).
  - The host ships each shard transposed ([D, 12544]) and fp8-e4m3 cast
    (f_norm side pre-scaled x16 so unit-norm entries stay well above the
    fp8 subnormal range; a uniform scale cannot change the ranking).
  - fp8 DoubleRow matmuls (2 d-chunks per instruction, 0.5 cyc/row)
    accumulate PSUM fp32 [128b, 512j] distance tiles.
  - ScalarE copies each PSUM tile to SBUF bf16; VectorE keeps a running
    elementwise max ("comb max") over the j-tiles in bf16 2x mode:
    comb[b, e] = max_t distance[b, t*512 + e], e in [0, 512).
  - VectorE max8 + find_index8 per 128-row chunk -> top-8 (comb max, e).
Host merges 8 cores x 8 candidate combs per row, recomputes each winning
comb's ~25 member distances in exact fp32 (so fp8/bf16 noise cannot affect
the final selection), takes the top-2 per comb (covers two neighbours
landing in one comb), re-ranks, drops the top-1 (the reference drops
idx[:,0]), gathers scores and reduces the loss.
"""

from contextlib import ExitStack

import numpy as np

import concourse.bass as bass
import concourse.tile as tile
from concourse import bacc, mybir
from concourse.bass_utils import run_bass_kernel_spmd

# Problem constants (hardcoded per contest rules).
B, D, N, C, K = 512, 512, 100000, 64, 5
EPS = 1e-12
NCORES = 8
NSHARD = 12544            # padded bank rows per core (98 * 128)
NPAD = NSHARD * NCORES    # 100352
BCH = 4                   # batch chunks of 128 rows
JT = 512                  # j-tile width == comb count per core
NJT = 25                  # 24 full tiles + one 256-wide tile
N_WARMUP = 8              # zero matmuls to warm the PE during the first DMA
FSCALE = 16.0             # f_norm pre-scale so fp8 quantization is well-conditioned

_F32 = mybir.dt.float32
_BF16 = mybir.dt.bfloat16
_FP8 = mybir.dt.float8e4
_U32 = mybir.dt.uint32

_cache = {}


def _build_module():
    nc = bacc.Bacc("TRN2", target_bir_lowering=False, debug=False,
                   num_devices=NCORES)
    # bank shard, transposed + fp8-cast on host: [D, NSHARD]
    fbt_d = nc.dram_tensor("fbt", [D, NSHARD], _FP8, kind="ExternalInput").ap()
    # f_norm.T (pre-scaled, fp8) packed on host as [dp, dc*B + b]
    fnt_d = nc.dram_tensor("fnt", [128, 4 * B], _FP8, kind="ExternalInput").ap()
    # top-8 combs per 128-row chunk: value and comb id
    val_out = nc.dram_tensor("val_out", [128, BCH, 8], _F32,
                             kind="ExternalOutput").ap()
    idx_out = nc.dram_tensor("idx_out", [128, BCH, 8], _U32,
                             kind="ExternalOutput").ap()

    with tile.TileContext(nc) as tc, ExitStack() as ctx:
        const = ctx.enter_context(tc.tile_pool(name="const", bufs=1))
        fbt_pool = ctx.enter_context(tc.tile_pool(name="fbt", bufs=4))
        dp_pool = ctx.enter_context(tc.tile_pool(name="dp", bufs=2, space="PSUM"))
        tmp_pool = ctx.enter_context(tc.tile_pool(name="tmp", bufs=3))
        out_pool = ctx.enter_context(tc.tile_pool(name="outs", bufs=1))

        # PE warm-up: harmless zero matmuls that run while the first DMAs land
        wu_sb = const.tile([128, JT], _F32)
        nc.gpsimd.memset(wu_sb[:], 0.0)
        wu_ps = dp_pool.tile([128, BCH, JT], _F32, tag="dp")
        wu_r = wu_sb[:].bitcast(_FP8).rearrange("p (c j) -> p c j", c=4)
        for _ in range(N_WARMUP):
            nc.tensor.matmul(wu_ps[:, 0], lhsT=wu_r[:, 0:2, :128], rhs=wu_r[:, 0:2],
                             start=True, stop=True,
                             perf_mode=mybir.MatmulPerfMode.DoubleRow)

        fnt_sb = const.tile([128, 4, B], _FP8)
        nc.sync.dma_start(fnt_sb[:], fnt_d.rearrange("p (c b) -> p c b", c=4))

        # running comb maxima, bf16, all four chunks in one tile
        acc = const.tile([128, BCH, JT], _BF16)
        vcat = out_pool.tile([128, BCH, 8], _F32)
        icat = out_pool.tile([128, BCH, 8], _U32)

        for t in range(NJT):
            j0 = t * JT
            W = min(JT, NSHARD - j0)

            # bank tile in [d, j] layout: partition = d % 128, c = d // 128
            fbt = fbt_pool.tile([128, 4, JT], _FP8, tag="fbt")
            nc.sync.dma_start(
                fbt[:, :, :W],
                fbt_d[:, j0:j0 + W].rearrange("(c p) j -> p c j", p=128),
            )

            dp = dp_pool.tile([128, BCH, JT], _F32, tag="dp")
            for bc in range(BCH):
                for h in range(2):        # DoubleRow: two d-chunks per matmul
                    nc.tensor.matmul(
                        dp[:, bc, :W],
                        lhsT=fnt_sb[:, 2 * h:2 * h + 2, bc * 128:(bc + 1) * 128],
                        rhs=fbt[:, 2 * h:2 * h + 2, :W],
                        start=(h == 0), stop=(h == 1),
                        perf_mode=mybir.MatmulPerfMode.DoubleRow,
                    )
            if t == 0:
                nc.scalar.copy(out=acc[:, :, :W], in_=dp[:, :, :W])
            elif t % 5 == 2:
                # a few tiles fold straight from PSUM on the VectorE to keep
                # the ScalarE (copy) and VectorE (fold) loads balanced
                nc.vector.tensor_max(acc[:, :, :W], acc[:, :, :W], dp[:, :, :W])
            else:
                tmp = tmp_pool.tile([128, BCH, JT], _BF16, tag="tmp")
                nc.scalar.copy(out=tmp[:, :, :W], in_=dp[:, :, :W])
                nc.vector.tensor_max(acc[:, :, :W], acc[:, :, :W], tmp[:, :, :W])

        for bc in range(BCH):
            sl = acc[:, bc]
            nc.vector.max(out=vcat[:, bc], in_=sl)
            nc.vector.max_index(out=icat[:, bc], in_max=vcat[:, bc], in_values=sl)
        nc.sync.dma_start(val_out, vcat[:])
        nc.sync.dma_start(idx_out, icat[:])

    nc.compile()
    return nc


def _get_module():
    if "nc" not in _cache:
        _cache["nc"] = _build_module()
    return _cache["nc"]


def kernel(features, predictions, fea_bank, score_bank, trg_idx):
    features = np.asarray(features, dtype=np.float32)
    predictions = np.asarray(predictions, dtype=np.float32)
    fea_bank = np.asarray(fea_bank, dtype=np.float32)
    score_bank = np.asarray(score_bank, dtype=np.float32)
    trg_idx = np.asarray(trg_idx, dtype=np.int32)

    # ---- tiny host prologue (O(B*D)) ----
    sm = predictions - predictions.max(axis=1, keepdims=True)
    np.exp(sm, out=sm)
    sm /= sm.sum(axis=1, keepdims=True)                       # softmax_out [B,C]
    nrm = np.maximum(np.sqrt((features * features).sum(axis=1, keepdims=True)),
                     EPS)
    f_norm = features / nrm                                   # [B,D]

    # bank updates + padding
    fbp = np.zeros((NPAD, D), dtype=np.float32)
    fbp[:N] = fea_bank
    fbp[trg_idx] = f_norm
    sb = score_bank.copy()
    sb[trg_idx] = sm

    # f_norm.T (pre-scaled for fp8 conditioning) packed as [dp, dc*B + b]
    import ml_dtypes
    fp8 = ml_dtypes.float8_e4m3
    fnt = np.ascontiguousarray(
        (f_norm.T * FSCALE).reshape(4, 128, B).transpose(1, 0, 2)
        .reshape(128, 4 * B)).astype(fp8)

    nc = _get_module()
    in_maps = [
        {"fbt": np.ascontiguousarray(
             fbp[c * NSHARD:(c + 1) * NSHARD].T).astype(fp8),
         "fnt": fnt}
        for c in range(NCORES)
    ]
    res = run_bass_kernel_spmd(nc, in_maps, core_ids=list(range(NCORES)))

    # ---- host epilogue: merge candidate combs, resolve in fp32, loss ----
    # outputs are [128, BCH, 8]; row b = bc*128 + p
    vals = np.empty((B, NCORES * 8), np.float32)
    core = np.empty((B, NCORES * 8), np.int64)
    comb = np.empty((B, NCORES * 8), np.int64)
    for c, r in enumerate(res.results):
        vals[:, c * 8:(c + 1) * 8] = \
            r["val_out"].transpose(1, 0, 2).reshape(B, 8)
        comb[:, c * 8:(c + 1) * 8] = \
            r["idx_out"].astype(np.int64).transpose(1, 0, 2).reshape(B, 8)
        core[:, c * 8:(c + 1) * 8] = c

    # preselect by the (fp8/bf16-precision) device values, then re-rank the
    # short list with exact fp32 dots so quantization noise cannot leak in
    TOP = 12
    order = np.argsort(-vals, axis=1, kind="stable")[:, :TOP]
    top_core = np.take_along_axis(core, order, axis=1)        # [B, TOP]
    top_comb = np.take_along_axis(comb, order, axis=1)        # [B, TOP]

    tt = np.arange(NJT, dtype=np.int64)[None, None, :]
    pos_local = top_comb[:, :, None] + tt * JT                # [B, TOP, NJT]
    valid = pos_local < NSHARD
    rows = top_core[:, :, None] * NSHARD + np.minimum(pos_local, NSHARD - 1)
    vecs = fbp[rows.reshape(-1)].reshape(B, TOP, NJT, D)
    dots = np.einsum("rktd,rd->rkt", vecs, f_norm, optimize=True)
    dots = np.where(valid & (rows < N), dots, np.float32(-np.inf))

    # top-2 member rows per comb (two neighbours may share one comb)
    p2 = np.argsort(-dots, axis=2)[:, :, :2]                  # [B, TOP, 2]
    v2 = np.take_along_axis(dots, p2, axis=2).reshape(B, 2 * TOP)
    i2 = np.take_along_axis(rows, p2, axis=2).reshape(B, 2 * TOP)

    # order exactly like jax.lax.top_k: value desc, index asc on ties
    reorder = np.lexsort((i2, -v2), axis=1)
    top_idx = np.take_along_axis(i2, reorder, axis=1)

    idx_near = top_idx[:, 1:K + 1]                            # drop self slot 0
    score_near = sb[idx_near].astype(np.float64)              # [B,K,C]
    kl = score_near * (np.log(score_near) - sm[:, None, :].astype(np.float64))
    loss = kl.sum(axis=(1, 2)).mean()

    s64 = sm.astype(np.float64)
    neg_pred = (np.square(s64.sum(axis=0)).sum()
                - np.square(s64).sum()) / B

    return np.float32(loss + neg_pred)


# revision 31
# speedup vs baseline: 2.0342x; 1.0794x over previous
"""Trainium2 Bass kernel for nn_AaD_MAPU (retrieval kNN + KL attraction / dispersion loss).

Reference computation:
    softmax_out = softmax(predictions)                      [B,C]
    f_norm      = l2_normalize(features)                    [B,D]
    fb          = fea_bank with rows trg_idx <- f_norm      [N,D]
    sb          = score_bank with rows trg_idx <- softmax   [N,C]
    distance    = f_norm @ fb.T                             [B,N]
    idx         = top_k(distance, K+1); idx_near = idx[:,1:]
    score_near  = sb[idx_near]                              [B,K,C]
    loss        = sum(score_near * (log(score_near) - softmax[:,None,:])) / B
    neg_pred    = mean(rowsum(softmax @ softmax.T - diag))
    out         = loss + neg_pred

Device strategy (8 NeuronCores, bank rows sharded, d-major layout):
  - Pad bank to 100352 rows; core c owns rows [c*12544, (c+1)*12544).
  - The host ships each shard transposed ([D, 12544]) and fp8-e4m3 cast
    (f_norm side pre-scaled x16 so unit-norm entries stay well above the
    fp8 subnormal range; a uniform scale cannot change the ranking).
  - fp8 DoubleRow matmuls (2 d-chunks per instruction, 0.5 cyc/row)
    accumulate PSUM fp32 [128b, 512j] distance tiles.
  - ScalarE copies each PSUM tile to SBUF bf16; VectorE keeps a running
    elementwise max ("comb max") over the j-tiles in bf16 2x mode:
    comb[b, e] = max_t distance[b, t*512 + e], e in [0, 512).
  - VectorE max8 + find_index8 per 128-row chunk -> top-8 (comb max, e).
Host merges 8 cores x 8 candidate combs per row, recomputes each winning
comb's ~25 member distances in exact fp32 (so fp8/bf16 noise cannot affect
the final selection), takes the top-2 per comb (covers two neighbours
landing in one comb), re-ranks, drops the top-1 (the reference drops
idx[:,0]), gathers scores and reduces the loss.
"""

from contextlib import ExitStack

import numpy as np

import concourse.bass as bass
import concourse.tile as tile
from concourse import bacc, mybir
from concourse.bass_utils import run_bass_kernel_spmd

# Problem constants (hardcoded per contest rules).
B, D, N, C, K = 512, 512, 100000, 64, 5
EPS = 1e-12
NCORES = 8
NSHARD = 12544            # padded bank rows per core (98 * 128)
NPAD = NSHARD * NCORES    # 100352
BCH = 4                   # batch chunks of 128 rows
JT = 512                  # j-tile width == comb count per core
NJT = 25                  # 24 full tiles + one 256-wide tile
N_WARMUP = 8              # zero matmuls to warm the PE during the first DMA
FSCALE = 16.0             # f_norm pre-scale so fp8 quantization is well-conditioned

_F32 = mybir.dt.float32
_BF16 = mybir.dt.bfloat16
_FP8 = mybir.dt.float8e4
_U32 = mybir.dt.uint32

_cache = {}


def _build_module():
    nc = bacc.Bacc("TRN2", target_bir_lowering=False, debug=False,
                   num_devices=NCORES)
    # bank shard, transposed + fp8-cast on host: [D, NSHARD]
    fbt_d = nc.dram_tensor("fbt", [D, NSHARD], _FP8, kind="ExternalInput").ap()
    # f_norm.T (pre-scaled, fp8) packed on host as [dp, dc*B + b]
    fnt_d = nc.dram_tensor("fnt", [128, 4 * B], _FP8, kind="ExternalInput").ap()
    # top-8 combs per 128-row chunk: value and comb id
    val_out = nc.dram_tensor("val_out", [128, BCH, 8], _F32,
                             kind="ExternalOutput").ap()
    idx_out = nc.dram_tensor("idx_out", [128, BCH, 8], _U32,
                             kind="ExternalOutput").ap()

    with tile.TileContext(nc) as tc, ExitStack() as ctx:
        const = ctx.enter_context(tc.tile_pool(name="const", bufs=1))
        fbt_pool = ctx.enter_context(tc.tile_pool(name="fbt", bufs=4))
        dp_pool = ctx.enter_context(tc.tile_pool(name="dp", bufs=2, space="PSUM"))
        tmp_pool = ctx.enter_context(tc.tile_pool(name="tmp", bufs=3))
        out_pool = ctx.enter_context(tc.tile_pool(name="outs", bufs=1))

        # PE warm-up: harmless zero matmuls that run while the first DMAs land
        wu_sb = const.tile([128, JT], _F32)
        nc.gpsimd.memset(wu_sb[:], 0.0)
        wu_ps = dp_pool.tile([128, BCH, JT], _F32, tag="dp")
        wu_r = wu_sb[:].bitcast(_FP8).rearrange("p (c j) -> p c j", c=4)
        for _ in range(N_WARMUP):
            nc.tensor.matmul(wu_ps[:, 0], lhsT=wu_r[:, 0:2, :128], rhs=wu_r[:, 0:2],
                             start=True, stop=True,
                             perf_mode=mybir.MatmulPerfMode.DoubleRow)

        fnt_sb = const.tile([128, 4, B], _FP8)
        nc.sync.dma_start(fnt_sb[:], fnt_d.rearrange("p (c b) -> p c b", c=4))

        # running comb maxima, bf16; two parity-striped accumulators so the
        # ScalarE copy -> VectorE fold chains of consecutive tiles pipeline
        acc = [const.tile([128, BCH, JT], _BF16, name=f"acc{i}") for i in range(2)]
        vcat = out_pool.tile([128, BCH, 8], _F32)
        icat = out_pool.tile([128, BCH, 8], _U32)

        for t in range(NJT):
            j0 = t * JT
            W = min(JT, NSHARD - j0)

            # bank tile in [d, j] layout: partition = d % 128, c = d // 128
            fbt = fbt_pool.tile([128, 4, JT], _FP8, tag="fbt")
            nc.sync.dma_start(
                fbt[:, :, :W],
                fbt_d[:, j0:j0 + W].rearrange("(c p) j -> p c j", p=128),
            )

            dp = dp_pool.tile([128, BCH, JT], _F32, tag="dp")
            for bc in range(BCH):
                for h in range(2):        # DoubleRow: two d-chunks per matmul
                    nc.tensor.matmul(
                        dp[:, bc, :W],
                        lhsT=fnt_sb[:, 2 * h:2 * h + 2, bc * 128:(bc + 1) * 128],
                        rhs=fbt[:, 2 * h:2 * h + 2, :W],
                        start=(h == 0), stop=(h == 1),
                        perf_mode=mybir.MatmulPerfMode.DoubleRow,
                    )
            a = acc[t % 2][:, :, :W]
            if t < 2:
                nc.scalar.copy(out=a, in_=dp[:, :, :W])
            elif t % 5 == 2:
                # a few tiles fold straight from PSUM on the VectorE to keep
                # the ScalarE (copy) and VectorE (fold) loads balanced
                nc.vector.tensor_max(a, a, dp[:, :, :W])
            else:
                tmp = tmp_pool.tile([128, BCH, JT], _BF16, tag="tmp")
                nc.scalar.copy(out=tmp[:, :, :W], in_=dp[:, :, :W])
                nc.vector.tensor_max(a, a, tmp[:, :, :W])

        nc.vector.tensor_max(acc[0][:], acc[0][:], acc[1][:])
        for bc in range(BCH):
            sl = acc[0][:, bc]
            nc.vector.max(out=vcat[:, bc], in_=sl)
            nc.vector.max_index(out=icat[:, bc], in_max=vcat[:, bc], in_values=sl)
        nc.sync.dma_start(val_out, vcat[:])
        nc.sync.dma_start(idx_out, icat[:])

    nc.compile()
    return nc


def _get_module():
    if "nc" not in _cache:
        _cache["nc"] = _build_module()
    return _cache["nc"]


def kernel(features, predictions, fea_bank, score_bank, trg_idx):
    features = np.asarray(features, dtype=np.float32)
    predictions = np.asarray(predictions, dtype=np.float32)
    fea_bank = np.asarray(fea_bank, dtype=np.float32)
    score_bank = np.asarray(score_bank, dtype=np.float32)
    trg_idx = np.asarray(trg_idx, dtype=np.int32)

    # ---- tiny host prologue (O(B*D)) ----
    sm = predictions - predictions.max(axis=1, keepdims=True)
    np.exp(sm, out=sm)
    sm /= sm.sum(axis=1, keepdims=True)                       # softmax_out [B,C]
    nrm = np.maximum(np.sqrt((features * features).sum(axis=1, keepdims=True)),
                     EPS)
    f_norm = features / nrm                                   # [B,D]

    # bank updates + padding
    fbp = np.zeros((NPAD, D), dtype=np.float32)
    fbp[:N] = fea_bank
    fbp[trg_idx] = f_norm
    sb = score_bank.copy()
    sb[trg_idx] = sm

    # f_norm.T (pre-scaled for fp8 conditioning) packed as [dp, dc*B + b]
    import ml_dtypes
    fp8 = ml_dtypes.float8_e4m3
    fnt = np.ascontiguousarray(
        (f_norm.T * FSCALE).reshape(4, 128, B).transpose(1, 0, 2)
        .reshape(128, 4 * B)).astype(fp8)

    nc = _get_module()
    in_maps = [
        {"fbt": np.ascontiguousarray(
             fbp[c * NSHARD:(c + 1) * NSHARD].T).astype(fp8),
         "fnt": fnt}
        for c in range(NCORES)
    ]
    res = run_bass_kernel_spmd(nc, in_maps, core_ids=list(range(NCORES)))

    # ---- host epilogue: merge candidate combs, resolve in fp32, loss ----
    # outputs are [128, BCH, 8]; row b = bc*128 + p
    vals = np.empty((B, NCORES * 8), np.float32)
    core = np.empty((B, NCORES * 8), np.int64)
    comb = np.empty((B, NCORES * 8), np.int64)
    for c, r in enumerate(res.results):
        vals[:, c * 8:(c + 1) * 8] = \
            r["val_out"].transpose(1, 0, 2).reshape(B, 8)
        comb[:, c * 8:(c + 1) * 8] = \
            r["idx_out"].astype(np.int64).transpose(1, 0, 2).reshape(B, 8)
        core[:, c * 8:(c + 1) * 8] = c

    # preselect by the (fp8/bf16-precision) device values, then re-rank the
    # short list with exact fp32 dots so quantization noise cannot leak in
    TOP = 12
    order = np.argsort(-vals, axis=1, kind="stable")[:, :TOP]
    top_core = np.take_along_axis(core, order, axis=1)        # [B, TOP]
    top_comb = np.take_along_axis(comb, order, axis=1)        # [B, TOP]

    tt = np.arange(NJT, dtype=np.int64)[None, None, :]
    pos_local = top_comb[:, :, None] + tt * JT                # [B, TOP, NJT]
    valid = pos_local < NSHARD
    rows = top_core[:, :, None] * NSHARD + np.minimum(pos_local, NSHARD - 1)
    vecs = fbp[rows.reshape(-1)].reshape(B, TOP, NJT, D)
    dots = np.einsum("rktd,rd->rkt", vecs, f_norm, optimize=True)
    dots = np.where(valid & (rows < N), dots, np.float32(-np.inf))

    # top-2 member rows per comb (two neighbours may share one comb)
    p2 = np.argsort(-dots, axis=2)[:, :, :2]                  # [B, TOP, 2]
    v2 = np.take_along_axis(dots, p2, axis=2).reshape(B, 2 * TOP)
    i2 = np.take_along_axis(rows, p2, axis=2).reshape(B, 2 * TOP)

    # order exactly like jax.lax.top_k: value desc, index asc on ties
    reorder = np.lexsort((i2, -v2), axis=1)
    top_idx = np.take_along_axis(i2, reorder, axis=1)

    idx_near = top_idx[:, 1:K + 1]                            # drop self slot 0
    score_near = sb[idx_near].astype(np.float64)              # [B,K,C]
    kl = score_near * (np.log(score_near) - sm[:, None, :].astype(np.float64))
    loss = kl.sum(axis=(1, 2)).mean()

    s64 = sm.astype(np.float64)
    neg_pred = (np.square(s64.sum(axis=0)).sum()
                - np.square(s64).sum()) / B

    return np.float32(loss + neg_pred)
